# revision 19
# baseline (speedup 1.0000x reference)
"""MoE gate (nn_MoEGate) Trainium2 Bass kernel.

Strategy (data-parallel over tokens, 8 cores):
  - Host: flatten x to [16384, 2048], transpose to [D, TOK] so the
    contraction dim lands on SBUF partitions, shard tokens 8 ways.
  - Device per core (2048 tokens, 16 tiles of 128):
      * FP32 matmul x.T-block-stationary: logits[t, e] in PSUM [128, 64]
      * ACT: exp(logits) -> SBUF, with accumulated per-token sum Z
      * DVE: m = #(logits >= null_logit) via tensor_scalar is_ge + accum
      * DVE: top-16 values+indices via max / max_index / match_replace
      * PE:  P_real partial = exp.T @ (1/Z)  (cross-partition reduce)
  - Host: merge real top-k with the 64 identical null experts
    (all nulls share logit null_logit, so top-k of the 128-way concat is
    [reals >= null in sorted order] ++ [null 64, 65, ...]), renormalize
    weights, bincount, and assemble the aux loss.
"""

import os

import numpy as np

import concourse.bass as bass
import concourse.mybir as mybir
from concourse import bacc
from concourse.bass_utils import run_bass_kernel_spmd
from concourse.tile import TileContext

# Problem constants (fixed by the grading harness).
B, T, D = 4, 4096, 2048
E, NULL, K = 64, 64, 10
RHO = 0.5
N_CORES = 8
TOK = B * T              # 16384 tokens
TPC = TOK // N_CORES     # 2048 tokens per core
P = 128                  # tokens per tile (SBUF partitions)
NTILES = TPC // P        # 16
KB = D // P              # 16 contraction blocks

f32 = mybir.dt.float32
u32 = mybir.dt.uint32

# Set by the last kernel() call when BASS_KERNEL_TRACE=1 (for test.py).
last_results = None


def _ensure_ntff_hook():
    """Register the axon NTFF profile hook if the antenv stub lacks it.

    trn_boot registers the hook only when ``antenv.axon_hooks`` is
    importable; the container ships a stub antenv without it. Inject an
    equivalent module and build the hook from the injected .so directly.
    """
    import sys
    import types

    try:
        from antenv.axon_hooks import get_axon_ntff_profile_hook  # noqa: F401
        return True
    except ImportError:
        pass
    try:
        import antenv
        from trn_agent_boot.trn_boot import _ntff_profile_via_ctypes

        hook = _ntff_profile_via_ctypes("/opt/axon/libaxon_pjrt.so")
        mod = types.ModuleType("antenv.axon_hooks")
        _state = {"hook": hook}
        mod.set_axon_ntff_profile_hook = lambda h: _state.__setitem__("hook", h)
        mod.get_axon_ntff_profile_hook = lambda: _state["hook"]
        sys.modules["antenv.axon_hooks"] = mod
        antenv.axon_hooks = mod
        return hook is not None
    except Exception:
        return False


def _build(null_logit: float, has_bias: bool):
    nc = bacc.Bacc(
        "TRN2",
        target_bir_lowering=False,
        debug=False,
        enable_asserts=True,
        num_devices=N_CORES,
    )
    xt = nc.dram_tensor("xt", [D, TPC], f32, kind="ExternalInput")
    wt = nc.dram_tensor("wt", [D, E], f32, kind="ExternalInput")
    if has_bias:
        bias_row = nc.dram_tensor("bias_row", [1, E], f32, kind="ExternalInput")
    out_v = nc.dram_tensor("out_v", [NTILES, P, 16], f32, kind="ExternalOutput")
    out_i = nc.dram_tensor("out_i", [NTILES, P, 16], u32, kind="ExternalOutput")
    out_z = nc.dram_tensor("out_z", [NTILES, P, 1], f32, kind="ExternalOutput")
    out_m = nc.dram_tensor("out_m", [NTILES, P, 1], f32, kind="ExternalOutput")
    out_p = nc.dram_tensor("out_p", [P, E], f32, kind="ExternalOutput")

    # d = k*128 + p for both operands so contraction pairs line up.
    xt_r = xt.rearrange("(k p) t -> p k t", p=P)   # [128, KB, TPC]
    wt_r = wt.rearrange("(k p) e -> p k e", p=P)   # [128, KB, E]

    with TileContext(nc) as tc:
        with (
            tc.tile_pool(name="wpool", bufs=1) as wpool,
            tc.tile_pool(name="xpool", bufs=3) as xpool,
            tc.tile_pool(name="spool", bufs=3) as spool,
            tc.tile_pool(name="opool", bufs=3) as opool,
            tc.tile_pool(name="psum", bufs=3, space="PSUM") as psum_pool,
        ):
            w_sb = wpool.tile([P, KB, E], f32, name="w_sb")
            nc.sync.dma_start(w_sb, wt_r)
            accp = wpool.tile([P, E], f32, name="accp")
            nc.vector.memset(accp, 0.0)
            if has_bias:
                ones_sb = wpool.tile([1, P], f32, name="ones_sb")
                nc.vector.memset(ones_sb, 1.0)
                bias_sb = wpool.tile([1, E], f32, name="bias_sb")
                nc.sync.dma_start(bias_sb, bias_row[:, :])


            for t in range(NTILES):
                x_sb = xpool.tile([P, KB, P], f32, name="x_sb")
                nc.sync.dma_start(x_sb, xt_r[:, :, t * P:(t + 1) * P])

                logits = psum_pool.tile([P, E], f32, name="logits")
                for k in range(KB):
                    nc.tensor.matmul(
                        logits,
                        x_sb[:, k, :],
                        w_sb[:, k, :],
                        start=(k == 0),
                        stop=(k == KB - 1 and not has_bias),
                    )
                if has_bias:
                    nc.tensor.matmul(logits, ones_sb, bias_sb, start=False, stop=True)

                exp_sb = spool.tile([P, E], f32, name="exp_sb")
                z = opool.tile([P, 1], f32, name="z")
                nc.scalar.activation(
                    exp_sb, logits, mybir.ActivationFunctionType.Exp, accum_out=z
                )
                # Exact f32 logits into SBUF for the top-k (ACT exp has ~1e-5
                # table error; ordering must come from the exact logits).
                logits_sb = spool.tile([P, E], f32, name="logits_sb")
                nc.scalar.copy(logits_sb, logits)

                scratch = spool.tile([P, E], f32, name="scratch")
                m = opool.tile([P, 1], f32, name="m")
                nc.vector.tensor_single_scalar(
                    scratch, logits, float(null_logit), mybir.AluOpType.is_ge
                )
                nc.vector.reduce_sum(m, scratch, axis=mybir.AxisListType.X)

                v16 = opool.tile([P, 16], f32, name="v16")
                i16 = opool.tile([P, 16], u32, name="i16")
                work2 = spool.tile([P, E], f32, name="work2")
                nc.vector.max(out=v16[:, 0:8], in_=logits_sb)
                nc.vector.max_index(out=i16[:, 0:8], in_max=v16[:, 0:8], in_values=logits_sb)
                nc.vector.match_replace(
                    out=work2, in_to_replace=v16[:, 0:8], in_values=logits_sb,
                    imm_value=-1e30,
                )
                nc.vector.max(out=v16[:, 8:16], in_=work2)
                nc.vector.max_index(out=i16[:, 8:16], in_max=v16[:, 8:16], in_values=work2)

                rz = opool.tile([P, 1], f32, name="rz")
                nc.vector.reciprocal(rz, z)
                nc.vector.scalar_tensor_tensor(
                    out=accp,
                    in0=exp_sb,
                    scalar=rz,
                    in1=accp,
                    op0=mybir.AluOpType.mult,
                    op1=mybir.AluOpType.add,
                )

                nc.sync.dma_start(out_v[t, :, :], v16)
                nc.sync.dma_start(out_i[t, :, :], i16)
                nc.sync.dma_start(out_z[t, :, :], z)
                nc.sync.dma_start(out_m[t, :, :], m)

            nc.sync.dma_start(out_p[:, :], accp)
    nc.finalize()
    return nc


def kernel(x, gate_w, logit_bias, null_logit):
    global last_results
    x = np.asarray(x, dtype=np.float32)
    gate_w = np.asarray(gate_w, dtype=np.float32)
    logit_bias = np.asarray(logit_bias, dtype=np.float32).reshape(E)
    null_f = float(np.asarray(null_logit))
    has_bias = bool(np.any(logit_bias != 0.0))

    xt = np.ascontiguousarray(x.reshape(TOK, D).T)   # [D, TOK]
    wt = np.ascontiguousarray(gate_w.T)              # [D, E]

    nc = _build(null_f, has_bias)
    in_maps = []
    for c in range(N_CORES):
        im = {"xt": np.ascontiguousarray(xt[:, c * TPC:(c + 1) * TPC]), "wt": wt}
        if has_bias:
            im["bias_row"] = logit_bias.reshape(1, E)
        in_maps.append(im)

    trace = os.environ.get("BASS_KERNEL_TRACE", "0") == "1"
    if trace:
        trace = _ensure_ntff_hook()
        try:
            # Artifact upload has no bucket in this container; neuter it.
            import concourse.bass_utils as _bu

            _bu.upload_artifacts = lambda tmpdir: tmpdir
        except Exception:
            pass
    res = run_bass_kernel_spmd(
        nc, in_maps, core_ids=list(range(N_CORES)), trace=trace
    )
    last_results = res
    rs = res.results

    v = np.concatenate([r["out_v"].reshape(TPC, 16) for r in rs])[:, :K]
    i = np.concatenate([r["out_i"].reshape(TPC, 16) for r in rs])[:, :K]
    z = np.concatenate([r["out_z"].reshape(TPC) for r in rs]).astype(np.float64)
    m_cnt = np.concatenate([r["out_m"].reshape(TPC) for r in rs]).astype(np.float64)
    p_partial = np.stack(
        [r["out_p"].astype(np.float64).sum(axis=0) for r in rs]
    )  # [N_CORES, E]

    # Host-side merge of real top-k with the identical-valued null experts.
    C = float(NULL) * float(np.exp(np.float64(null_f)))
    jj = np.arange(K, dtype=np.float64)[None, :]
    mm = m_cnt[:, None]
    valid = jj < mm                                   # slot is a real expert
    nullidx = (E + jj - mm)
    idx = np.where(valid, i.astype(np.int64), nullidx.astype(np.int64))
    ev = np.exp(v.astype(np.float64))                 # v holds top-10 logits
    evmask = np.where(valid, ev, 0.0)
    wsum = evmask.sum(-1)
    zfull = z + C
    denom = np.maximum(wsum, 1e-6 * zfull)
    topk_w = (evmask / denom[:, None]).astype(np.float32)
    is_null = ~valid

    P_real = p_partial.sum(0) / TOK
    counts = np.bincount(idx[valid], minlength=E).astype(np.float64)[:E]
    f_real = counts / np.clip(counts.sum(), 1e-6, None)
    L_bal = E * float((f_real * P_real).sum())
    lse = np.log(zfull)
    L_z = float(np.mean(lse ** 2))
    null_rate = float(is_null.mean())
    L_null = (null_rate - RHO) ** 2
    aux = np.float32(0.02 * L_bal + 0.001 * L_z + 0.01 * L_null)

    return (
        idx.reshape(B, T, K).astype(np.int32),
        topk_w.reshape(B, T, K),
        is_null.reshape(B, T, K),
        aux,
    )
